# revision 3
# baseline (speedup 1.0000x reference)
"""Label-wise FFN kernel for Trainium2 (8 NeuronCores, label-sharded).

Computes out[b, l] = relu(x @ W1[l] + b1[l]) @ W2[l] + b2[l] for
B=8192, D=1024, L=64, H=256, fp32 in/out.

Sharding: L is split across the 8 cores (8 labels each); every core holds a
full replica of x. Each core runs both layers for its labels and writes its
[8, B] slice of the (transposed) output; the host concatenates and
transposes back to [B, L].

Per-core dataflow (all matmuls in float32r = TF32-like single-pass PE mode):
  layer 1: psum[h=128, b=512] += W1tile[d=128, h=128].T @ xT[d=128, b=512]
           over 8 d-tiles; ACT fuses bias-add (per-partition) + ReLU on the
           PSUM->SBUF copy.
  layer 2: psum2[8, b=512] += W2blk[h=128, 8].T @ h[h=128, b=512] over the
           16 (label, h-chunk) tiles, with W2blk block-diagonal so each
           output partition is one label's dot product; ACT adds b2.
"""

import numpy as np

import concourse.bacc as bacc
import concourse.mybir as mybir
import concourse.tile as tile
from concourse.bass_utils import run_bass_kernel_spmd

B, D, L, H = 8192, 1024, 64, 256
NCORES = 8
LPC = L // NCORES      # labels per core
P = 128
HC = H // P            # h-chunks per label
KT = D // P            # k-tiles over D
BCHUNK = 512
NB = B // BCHUNK       # b-chunks
NL2 = LPC * HC         # layer-2 k-tiles


def build_nc():
    f32r = mybir.dt.float32r
    f32 = mybir.dt.float32
    nc = bacc.Bacc(None, target_bir_lowering=False)

    xT = nc.dram_tensor("xT", [D, B], f32r, kind="ExternalInput")
    w1t = nc.dram_tensor("w1t", [LPC, HC, KT, P, P], f32r, kind="ExternalInput")
    w2blk = nc.dram_tensor("w2blk", [NL2, P, LPC], f32r, kind="ExternalInput")
    b1t = nc.dram_tensor("b1t", [NL2, P], f32, kind="ExternalInput")
    b2c = nc.dram_tensor("b2c", [LPC, 1], f32, kind="ExternalInput")
    out = nc.dram_tensor("out", [LPC, B], f32, kind="ExternalOutput")

    relu = mybir.ActivationFunctionType.Relu
    ident = mybir.ActivationFunctionType.Identity

    with tile.TileContext(nc) as tc:
        with (
            tc.tile_pool(name="wpool", bufs=1) as wpool,
            tc.tile_pool(name="xpool", bufs=16) as xpool,
            tc.tile_pool(name="hpool", bufs=4) as hpool,
            tc.tile_pool(name="opool", bufs=4) as opool,
            tc.tile_pool(name="ps1", bufs=5, space="PSUM") as ps1pool,
            tc.tile_pool(name="ps2", bufs=2, space="PSUM") as ps2pool,
        ):
            # Resident weights/biases.
            w1sb = []
            for l in range(LPC):
                t = wpool.tile([P, HC, KT, P], f32r, tag=f"w1_{l}")
                for hc in range(HC):
                    nc.sync.dma_start(
                        t[:, hc], w1t[l, hc].rearrange("k i j -> i k j")
                    )
                w1sb.append(t)
            w2sb = wpool.tile([P, NL2, LPC], f32r, tag="w2")
            nc.sync.dma_start(w2sb[:], w2blk.rearrange("n p j -> p n j"))
            b1sb = wpool.tile([P, NL2], f32, tag="b1")
            nc.sync.dma_start(b1sb[:], b1t.rearrange("n p -> p n"))
            b2sb = wpool.tile([LPC, 1], f32, tag="b2")
            nc.sync.dma_start(b2sb[:], b2c[:])

            xT_t = xT.rearrange("(k p) b -> p k b", p=P)

            for c in range(NB):
                bs = c * BCHUNK
                xk = []
                for kt in range(KT):
                    t = xpool.tile([P, BCHUNK], f32r, tag="xk")
                    nc.sync.dma_start(t[:], xT_t[:, kt, bs : bs + BCHUNK])
                    xk.append(t)
                ps2 = ps2pool.tile([LPC, BCHUNK], f32)
                for l in range(LPC):
                    for hc in range(HC):
                        idx = l * HC + hc
                        ps1 = ps1pool.tile([P, BCHUNK], f32)
                        for kt in range(KT):
                            nc.tensor.matmul(
                                ps1[:],
                                w1sb[l][:, hc, kt],
                                xk[kt][:],
                                start=(kt == 0),
                                stop=(kt == KT - 1),
                            )
                        ht = hpool.tile([P, BCHUNK], f32r)
                        nc.scalar.activation(
                            ht[:], ps1[:], relu, bias=b1sb[:, idx : idx + 1]
                        )
                        nc.tensor.matmul(
                            ps2[:],
                            w2sb[:, idx],
                            ht[:],
                            start=(idx == 0),
                            stop=(idx == NL2 - 1),
                        )
                ot = opool.tile([LPC, BCHUNK], f32)
                nc.scalar.activation(ot[:], ps2[:], ident, bias=b2sb[:, 0:1])
                nc.sync.dma_start(out[:, bs : bs + BCHUNK], ot[:])

    nc.compile()
    return nc


def make_in_maps(x, W1, b1, W2, b2):
    """Shard + lay out the full inputs into per-core input maps."""
    x = np.asarray(x, dtype=np.float32)
    W1 = np.asarray(W1, dtype=np.float32)
    b1 = np.asarray(b1, dtype=np.float32)
    W2 = np.asarray(W2, dtype=np.float32)
    b2 = np.asarray(b2, dtype=np.float32)

    xT = np.ascontiguousarray(x.T)  # [D, B], shared replica
    in_maps = []
    for core in range(NCORES):
        sl = slice(core * LPC, (core + 1) * LPC)
        w1s = W1[sl]  # [LPC, D, H]
        # [LPC, D, H] -> [LPC, HC, KT, 128(d), 128(h)]
        w1tile = np.ascontiguousarray(
            w1s.reshape(LPC, KT, P, HC, P).transpose(0, 3, 1, 2, 4)
        )
        w2s = W2[sl]  # [LPC, H]
        w2b = np.zeros((NL2, P, LPC), dtype=np.float32)
        for l in range(LPC):
            for hc in range(HC):
                w2b[l * HC + hc, :, l] = w2s[l, hc * P : (hc + 1) * P]
        b1s = b1[sl]  # [LPC, H]
        b1tile = np.ascontiguousarray(b1s.reshape(NL2, P))
        b2s = np.ascontiguousarray(b2[sl].reshape(LPC, 1))
        in_maps.append(
            {
                "xT": xT,
                "w1t": w1tile,
                "w2blk": w2b,
                "b1t": b1tile,
                "b2c": b2s,
            }
        )
    return in_maps


def kernel(x, W1, b1, W2, b2):
    nc = build_nc()
    in_maps = make_in_maps(x, W1, b1, W2, b2)
    res = run_bass_kernel_spmd(nc, in_maps, core_ids=list(range(NCORES)))
    outs = [res.results[c]["out"] for c in range(NCORES)]  # each [LPC, B]
    full = np.concatenate(outs, axis=0)  # [L, B]
    return np.ascontiguousarray(full.T).astype(np.float32)  # [B, L]


# revision 6
# speedup vs baseline: 42.6766x; 42.6766x over previous
"""Label-wise FFN kernel for Trainium2 (8 NeuronCores, label-sharded).

Computes out[b, l] = relu(x @ W1[l] + b1[l]) @ W2[l] + b2[l] for
B=8192, D=1024, L=64, H=256, fp32 in/out.

Sharding: L is split across the 8 cores (8 labels each); every core holds a
full replica of x. Each core runs both layers for its labels and writes its
[8, B] slice of the (transposed) output; the host concatenates and
transposes back to [B, L].

Per-core dataflow (all matmuls in float32r = TF32-like single-pass PE mode):
  layer 1: psum[h=128, b=512] += W1tile[d=128, h=128].T @ xT[d=128, b=512]
           over 8 d-tiles; ACT fuses bias-add (per-partition) + ReLU on the
           PSUM->SBUF copy.
  layer 2: psum2[8, b=512] += W2blk[h=128, 8].T @ h[h=128, b=512] over the
           16 (label, h-chunk) tiles, with W2blk block-diagonal so each
           output partition is one label's dot product; ACT adds b2.

Measured on HW (axon, For_i x64 amplification): ~611 us/core/pass,
absmax/scale ~2.4e-4 vs the fp32 reference. The f32r fused weight load
caps the sustained matmul rate at ~280 ns per [128x128]x[128x512] MM
(vs 213 ns pure-stream), which this schedule saturates.
"""

import numpy as np

import concourse.bacc as bacc
import concourse.mybir as mybir
import concourse.tile as tile
from concourse.bass_utils import run_bass_kernel_spmd

B, D, L, H = 8192, 1024, 64, 256
NCORES = 8
LPC = L // NCORES      # labels per core
P = 128
HC = H // P            # h-chunks per label
KT = D // P            # k-tiles over D
BCHUNK = 512
NB = B // BCHUNK       # b-chunks
NL2 = LPC * HC         # layer-2 k-tiles


def build_nc():
    f32r = mybir.dt.float32r
    f32 = mybir.dt.float32
    nc = bacc.Bacc(None, target_bir_lowering=False)

    xT = nc.dram_tensor("xT", [D, B], f32r, kind="ExternalInput")
    w1t = nc.dram_tensor("w1t", [LPC, HC, KT, P, P], f32r, kind="ExternalInput")
    w2blk = nc.dram_tensor("w2blk", [NL2, P, LPC], f32r, kind="ExternalInput")
    b1t = nc.dram_tensor("b1t", [NL2, P], f32, kind="ExternalInput")
    b2c = nc.dram_tensor("b2c", [LPC, 1], f32, kind="ExternalInput")
    out = nc.dram_tensor("out", [LPC, B], f32, kind="ExternalOutput")

    relu = mybir.ActivationFunctionType.Relu
    ident = mybir.ActivationFunctionType.Identity

    with tile.TileContext(nc) as tc:
        with (
            tc.tile_pool(name="wpool", bufs=1) as wpool,
            tc.tile_pool(name="xpool", bufs=16) as xpool,
            tc.tile_pool(name="hpool", bufs=18) as hpool,
            tc.tile_pool(name="opool", bufs=4) as opool,
            tc.tile_pool(name="ps1", bufs=5, space="PSUM") as ps1pool,
            tc.tile_pool(name="ps2", bufs=2, space="PSUM") as ps2pool,
        ):
            # Resident weights/biases.
            w1sb = []
            for l in range(LPC):
                t = wpool.tile([P, HC, KT, P], f32r, tag=f"w1_{l}")
                for hc in range(HC):
                    nc.sync.dma_start(
                        t[:, hc], w1t[l, hc].rearrange("k i j -> i k j")
                    )
                w1sb.append(t)
            w2sb = wpool.tile([P, NL2, LPC], f32r, tag="w2")
            nc.sync.dma_start(w2sb[:], w2blk.rearrange("n p j -> p n j"))
            b1sb = wpool.tile([P, NL2], f32, tag="b1")
            nc.sync.dma_start(b1sb[:], b1t.rearrange("n p -> p n"))
            b2sb = wpool.tile([LPC, 1], f32, tag="b2")
            nc.sync.dma_start(b2sb[:], b2c[:])

            xT_t = xT.rearrange("(k p) b -> p k b", p=P)

            for c in range(NB):
                bs = c * BCHUNK
                xk = []
                for kt in range(KT):
                    t = xpool.tile([P, BCHUNK], f32r, tag="xk")
                    nc.sync.dma_start(t[:], xT_t[:, kt, bs : bs + BCHUNK])
                    xk.append(t)
                ps2 = ps2pool.tile([LPC, BCHUNK], f32)
                hts = []
                for l in range(LPC):
                    for hc in range(HC):
                        idx = l * HC + hc
                        ps1 = ps1pool.tile([P, BCHUNK], f32)
                        for kt in range(KT):
                            nc.tensor.matmul(
                                ps1[:],
                                w1sb[l][:, hc, kt],
                                xk[kt][:],
                                start=(kt == 0),
                                stop=(kt == KT - 1),
                            )
                        ht = hpool.tile([P, BCHUNK], f32r)
                        nc.scalar.activation(
                            ht[:], ps1[:], relu, bias=b1sb[:, idx : idx + 1]
                        )
                        hts.append(ht)
                for idx, ht in enumerate(hts):
                    nc.tensor.matmul(
                        ps2[:],
                        w2sb[:, idx],
                        ht[:],
                        start=(idx == 0),
                        stop=(idx == NL2 - 1),
                    )
                ot = opool.tile([LPC, BCHUNK], f32)
                nc.scalar.activation(ot[:], ps2[:], ident, bias=b2sb[:, 0:1])
                nc.sync.dma_start(out[:, bs : bs + BCHUNK], ot[:])

    nc.compile()
    return nc


def make_in_maps(x, W1, b1, W2, b2):
    """Shard + lay out the full inputs into per-core input maps."""
    x = np.asarray(x, dtype=np.float32)
    W1 = np.asarray(W1, dtype=np.float32)
    b1 = np.asarray(b1, dtype=np.float32)
    W2 = np.asarray(W2, dtype=np.float32)
    b2 = np.asarray(b2, dtype=np.float32)

    xT = np.ascontiguousarray(x.T)  # [D, B], shared replica
    in_maps = []
    for core in range(NCORES):
        sl = slice(core * LPC, (core + 1) * LPC)
        w1s = W1[sl]  # [LPC, D, H]
        # [LPC, D, H] -> [LPC, HC, KT, 128(d), 128(h)]
        w1tile = np.ascontiguousarray(
            w1s.reshape(LPC, KT, P, HC, P).transpose(0, 3, 1, 2, 4)
        )
        w2s = W2[sl]  # [LPC, H]
        w2b = np.zeros((NL2, P, LPC), dtype=np.float32)
        for l in range(LPC):
            for hc in range(HC):
                w2b[l * HC + hc, :, l] = w2s[l, hc * P : (hc + 1) * P]
        b1s = b1[sl]  # [LPC, H]
        b1tile = np.ascontiguousarray(b1s.reshape(NL2, P))
        b2s = np.ascontiguousarray(b2[sl].reshape(LPC, 1))
        in_maps.append(
            {
                "xT": xT,
                "w1t": w1tile,
                "w2blk": w2b,
                "b1t": b1tile,
                "b2c": b2s,
            }
        )
    return in_maps


def kernel(x, W1, b1, W2, b2):
    nc = build_nc()
    in_maps = make_in_maps(x, W1, b1, W2, b2)
    res = run_bass_kernel_spmd(nc, in_maps, core_ids=list(range(NCORES)))
    outs = [res.results[c]["out"] for c in range(NCORES)]  # each [LPC, B]
    full = np.concatenate(outs, axis=0)  # [L, B]
    return np.ascontiguousarray(full.T).astype(np.float32)  # [B, L]
